# revision 13
# baseline (speedup 1.0000x reference)
"""ArcMargin head (ArcFace) distributed over 8 TRN2 NeuronCores.

Strategy (classification / tensor parallel), v4 — transposed-output design:
  - weight [C, D] sharded along C (12500 classes/core, padded to 12544);
    embeddings + labels replicated.
  - The host uploads the weight shard twice in bf16 (same total bytes as one
    f32 copy): once pre-TRANSPOSED [D, CSP] so the TensorEngine needs no
    on-chip weight transposes, once natural [CSP, D] for the row-norm
    computation (free-axis Square+accum) and for the per-label row gather.
  - The device computes the TRANSPOSED logits block out[c, b] = 64 *
    (w_c . e_hat_b):  lhsT = wT tile (classes stationary), rhs = eT.  With
    classes on PSUM partitions, the weight normalization folds into PSUM
    evacuation as a free per-partition scalar multiply (alternating
    Vector/Scalar engines), and the output is written as bf16 (halves the
    dominant HBM traffic; rel-err budget 2e-2 >> bf16 noise).
  - ArcFace margin: only the single target element per row changes.  The
    2048 target cosines come from an indirect row gather of weight[labels]
    plus a fused normalize-dot on GpSimd; the phi values are DMA'd out as a
    tiny [128, 16] tensor and placed into the full output during the host
    unshard step (all math stays on device; the host only does indexing).
  - Ramp: embeddings load as 16 small DMAs ahead of everything, the first
    weight block is split so the first matmuls can start at ~7 us, and the
    embedding-norm pipeline is spread across Scalar/Vector/GpSimd.
"""

import math
import sys

import numpy as np
import ml_dtypes

for _p in ("/opt/trn_rl_repo",):
    if _p not in sys.path:
        sys.path.append(_p)

import concourse.bass as bass
import concourse.tile as tile
from concourse import bacc
from concourse import mybir
from concourse.bass_utils import run_bass_kernel_spmd

SCALE = 64.0
MARGIN = 0.5
COS_M = math.cos(MARGIN)
SIN_M = math.sin(MARGIN)
TH = math.cos(math.pi - MARGIN)
MM = math.sin(math.pi - MARGIN) * MARGIN

B, D, C = 2048, 512, 100000
N_CORES = 8
CS = C // N_CORES          # 12500 real classes per core
CSP = 12544                # padded classes per core (98 * 128)
NJ = CSP // 128            # 98 class chunks
CB = 1792                  # weight-block width (7 blocks x 14 chunks)
NBLK = CSP // CB           # 7
JPB = CB // 128            # 14 chunks per block
OOB = 1 << 30              # gather offset sentinel for "not my row"

NPBF = ml_dtypes.bfloat16

F32 = mybir.dt.float32
BF16 = mybir.dt.bfloat16
I32 = mybir.dt.int32
AF = mybir.ActivationFunctionType
ALU = mybir.AluOpType


def build_program(b=B, d=D, csp=CSP):
    """Build the (SPMD-uniform) single-core Bass program."""
    mb = b // 128          # 16 batch row-chunks
    kc = d // 128          # 4 contraction chunks
    nc = bacc.Bacc()

    emb_d = nc.declare_dram_parameter("emb", [b, d], F32, isOutput=False)
    wt_d = nc.declare_dram_parameter("wt", [d, csp], BF16, isOutput=False)
    wn_d = nc.declare_dram_parameter("wn", [csp, d], BF16, isOutput=False)
    goff_d = nc.declare_dram_parameter("goff", [128, mb], I32, isOutput=False)
    ident_d = nc.declare_dram_parameter("ident", [128, 128], F32, isOutput=False)
    # flat transposed output [c * B + b]
    out_d = nc.declare_dram_parameter("out", [csp * b, 1], BF16, isOutput=True)
    tv_d = nc.declare_dram_parameter("tv", [128, mb], F32, isOutput=True)

    with tile.TileContext(nc) as tc:
        with (
            tc.tile_pool(name="const", bufs=1) as constp,
            tc.tile_pool(name="persist", bufs=1) as persist,
            tc.tile_pool(name="eld", bufs=16) as eldp,
            tc.tile_pool(name="wtp", bufs=3) as wtp,
            tc.tile_pool(name="wnp", bufs=3) as wnp,
            tc.tile_pool(name="scr", bufs=2) as scrp,
            tc.tile_pool(name="smp", bufs=4) as smp,
            tc.tile_pool(name="outp", bufs=3) as outp,
            tc.tile_pool(name="tpsum", bufs=2, space="PSUM") as tpsum,
            tc.tile_pool(name="cpsum", bufs=3, space="PSUM") as cpsum,
        ):
            ident = constp.tile([128, 128], BF16)
            nc.gpsimd.dma_start(out=ident[:], in_=ident_d[:])  # SWDGE casts f32->bf16
            zb = constp.tile([128, 1], F32, tag="zb")
            nc.vector.memset(zb[:], 0.0)
            epsb = constp.tile([128, 1], F32, tag="epsb")
            nc.vector.memset(epsb[:], 1e-24)
            s2b = constp.tile([128, 1], F32, tag="s2b")
            nc.vector.memset(s2b[:], SCALE * SCALE)

            eT = persist.tile([128, kc, b], BF16)       # (64*e_hat)^T
            ebf_all = persist.tile([128, mb, d], BF16)  # 64*e_hat, natural layout
            nsq = persist.tile([128, NJ], F32)          # per-class sum(w^2)
            nrm = persist.tile([128, NJ], F32)
            rn = persist.tile([128, NJ], F32)           # 1/||w_c||
            svec = persist.tile([128, mb], F32)         # 64*cos(target)
            tval = persist.tile([128, mb], F32)         # 64*phi / else-branch
            gofft = persist.tile([128, mb], I32)

            outv = out_d[:].rearrange("(c b) o -> c (b o)", b=b)  # [csp, b]

            # ---------------- DMA helpers ----------------
            def eg_m(m):
                t = eldp.tile([128, 1, d], F32, tag="eg", name=f"eg_{m}")
                nc.sync.dma_start(
                    out=t[:, 0, :],
                    in_=emb_d[m * 128:(m + 1) * 128, :],
                )
                return t

            def wt_blk(blk):
                t = wtp.tile([128, kc, CB], BF16, tag="wt", name=f"wt_{blk}")
                nc.sync.dma_start(
                    out=t[:],
                    in_=wt_d[:, blk * CB:(blk + 1) * CB].rearrange(
                        "(k p) c -> p k c", p=128
                    ),
                )
                return t

            wn_tiles = {}

            def wn_g(g):
                r0 = g * 512
                ng = min(4, NJ - g * 4)
                t = wnp.tile([128, 4, d], BF16, tag="wn", name=f"wn_{g}")
                nc.sync.dma_start(
                    out=t[:, :ng, :],
                    in_=wn_d[r0:r0 + ng * 128, :].rearrange(
                        "(g2 p) dd -> p g2 dd", p=128
                    ),
                )
                wn_tiles[g] = t

            # ---------------- compute helpers ----------------
            def e_norm(m, eg):
                et = eg[:, 0, :]
                ssq = smp.tile([128, 1], F32, tag="ssqe")
                r = m % 3
                if r == 0:
                    sq = scrp.tile([128, d], BF16, tag="sqe")
                    nc.scalar.activation(
                        out=sq[:], in_=et, func=AF.Square, bias=zb[:],
                        accum_out=ssq[:],
                    )
                elif r == 1:
                    sq = scrp.tile([128, d], BF16, tag="sqe1")
                    nc.vector.scalar_tensor_tensor(
                        out=sq[:], in0=et, scalar=1.0, in1=et,
                        op0=ALU.mult, op1=ALU.mult, accum_out=ssq[:],
                    )
                else:
                    # Pool engine: squares only; free-axis reduce is DVE-only
                    sq = scrp.tile([128, d], BF16, tag="sqe2")
                    nc.gpsimd.tensor_tensor(
                        out=sq[:], in0=et, in1=et, op=ALU.mult
                    )
                    nc.vector.tensor_reduce(
                        out=ssq[:], in_=sq[:], axis=mybir.AxisListType.X,
                        op=ALU.add,
                    )
                nrm_e = smp.tile([128, 1], F32, tag="nrme")
                nc.scalar.activation(out=nrm_e[:], in_=ssq[:], func=AF.Sqrt, bias=epsb[:])
                rec = smp.tile([128, 1], F32, tag="rece")
                nc.vector.reciprocal(out=rec[:], in_=nrm_e[:])
                # ebf = (e * (1/||e||)) * 64, fused two-scalar op (DVE only:
                # TensorScalarPtr is not available on the Pool engine)
                nc.vector.tensor_scalar(
                    out=ebf_all[:, m, :], in0=et, scalar1=rec[:], scalar2=SCALE,
                    op0=ALU.mult, op1=ALU.mult,
                )

            def e_transpose(m):
                for k in range(kc):
                    pt = tpsum.tile([128, 128], BF16)
                    nc.tensor.transpose(
                        out=pt[:], in_=ebf_all[:, m, k * 128:(k + 1) * 128],
                        identity=ident[:],
                    )
                    if (m * kc + k) % 2 == 0:
                        nc.scalar.copy(
                            out=eT[:, k, m * 128:(m + 1) * 128], in_=pt[:]
                        )
                    else:
                        nc.vector.tensor_copy(
                            out=eT[:, k, m * 128:(m + 1) * 128], in_=pt[:]
                        )

            def wnorm_chunk(c):
                sq = scrp.tile([128, d], BF16, tag="sqw")
                nc.scalar.activation(
                    out=sq[:], in_=wn_tiles[c // 4][:, c % 4, :], func=AF.Square,
                    bias=zb[:], accum_out=nsq[:, c:c + 1],
                )

            def rn_fin(g):
                s0 = g * 4
                s1 = min(s0 + 4, NJ)
                nc.scalar.activation(
                    out=nrm[:, s0:s1], in_=nsq[:, s0:s1], func=AF.Sqrt, bias=epsb[:]
                )
                nc.vector.reciprocal(out=rn[:, s0:s1], in_=nrm[:, s0:s1])

            def phase_wsel(m):
                # gather this chunk's owned target weight rows from the shard
                wsld = scrp.tile([128, d], BF16, tag="wsld", name=f"ws_{m}")
                nc.gpsimd.indirect_dma_start(
                    out=wsld[:],
                    out_offset=None,
                    in_=wn_d[:],
                    in_offset=bass.IndirectOffsetOnAxis(
                        ap=gofft[:, m:m + 1], axis=0
                    ),
                    bounds_check=csp - 1,
                    oob_is_err=False,
                )
                sq = scrp.tile([128, d], BF16, tag="sqs")
                ssq = smp.tile([128, 1], F32, tag="ssqs")
                nc.gpsimd.tensor_tensor(
                    out=sq[:], in0=wsld[:], in1=wsld[:], op=ALU.mult
                )
                nc.vector.tensor_reduce(
                    out=ssq[:], in_=sq[:], axis=mybir.AxisListType.X, op=ALU.add
                )
                nrm_s = smp.tile([128, 1], F32, tag="nrms")
                nc.scalar.activation(out=nrm_s[:], in_=ssq[:], func=AF.Sqrt, bias=epsb[:])
                rec = smp.tile([128, 1], F32, tag="recs")
                nc.vector.reciprocal(out=rec[:], in_=nrm_s[:])
                # dot = sum(wsld * ebf) on gpsimd, then svec = dot * (1/||w||)
                ttr = scrp.tile([128, d], BF16, tag="ttr")
                dotm = smp.tile([128, 1], F32, tag="dotm")
                nc.gpsimd.tensor_tensor(
                    out=ttr[:], in0=wsld[:], in1=ebf_all[:, m, :], op=ALU.mult
                )
                nc.vector.tensor_reduce(
                    out=dotm[:], in_=ttr[:], axis=mybir.AxisListType.X, op=ALU.add
                )
                nc.vector.tensor_tensor(
                    out=svec[:, m:m + 1], in0=dotm[:], in1=rec[:], op=ALU.mult
                )

            def phi_block():
                s2 = smp.tile([128, mb], F32, tag="s2")
                nc.scalar.activation(
                    out=s2[:], in_=svec[:], func=AF.Square, bias=zb[:]
                )
                rl = smp.tile([128, mb], F32, tag="rl")
                nc.scalar.activation(
                    out=rl[:], in_=s2[:], func=AF.Relu, bias=s2b[:], scale=-1.0
                )
                sn = smp.tile([128, mb], F32, tag="sn")
                nc.scalar.activation(out=sn[:], in_=rl[:], func=AF.Sqrt, bias=zb[:])
                pc = smp.tile([128, mb], F32, tag="pc")
                nc.vector.tensor_scalar_mul(out=pc[:], in0=svec[:], scalar1=COS_M)
                smt = smp.tile([128, mb], F32, tag="smt")
                nc.vector.tensor_scalar_mul(out=smt[:], in0=sn[:], scalar1=SIN_M)
                ph = smp.tile([128, mb], F32, tag="ph")
                nc.vector.tensor_tensor(
                    out=ph[:], in0=pc[:], in1=smt[:], op=ALU.subtract
                )
                eb = smp.tile([128, mb], F32, tag="eb")
                nc.vector.tensor_scalar_add(
                    out=eb[:], in0=svec[:], scalar1=-SCALE * MM
                )
                mk = smp.tile([128, mb], mybir.dt.uint8, tag="mk")
                nc.vector.tensor_scalar(
                    out=mk[:], in0=svec[:], scalar1=SCALE * TH, scalar2=None,
                    op0=ALU.is_gt,
                )
                nc.vector.select(out=tval[:], mask=mk[:], on_true=ph[:], on_false=eb[:])
                nc.sync.dma_start(out=tv_d[:], in_=tval[:])

            # ---------------- prologue (DMA order = ring order) ----------------
            eg_tiles = [None] * mb
            for m in range(8):
                eg_tiles[m] = eg_m(m)
            wn_g(0)
            # split first weight block: j0/j1 lhsT available early
            wt0a = wtp.tile([128, kc, 256], BF16, tag="wt0a")
            nc.sync.dma_start(
                out=wt0a[:],
                in_=wt_d[:, 0:256].rearrange("(k p) c -> p k c", p=128),
            )
            for m in range(8, mb):
                eg_tiles[m] = eg_m(m)
            wt0b = wtp.tile([128, kc, CB - 256], BF16, tag="wt0b")
            nc.sync.dma_start(
                out=wt0b[:],
                in_=wt_d[:, 256:CB].rearrange("(k p) c -> p k c", p=128),
            )
            wn_g(1)
            nc.sync.dma_start(out=gofft[:], in_=goff_d[:])

            wdone = 0
            for m in range(mb):
                e_norm(m, eg_tiles[m])
                e_transpose(m)
                if m % 4 == 3 and wdone < 8:
                    wnorm_chunk(wdone)
                    wnorm_chunk(wdone + 1)
                    wdone += 2
                    if wdone % 4 == 0:
                        rn_fin(wdone // 4 - 1)

            # ---------------- main loop over class chunks ----------------
            cur_wt = None
            nxt_wt = None
            wsel_done = 0
            for j in range(NJ):
                blk, jj = divmod(j, JPB)
                if jj == 0:
                    if blk > 0:
                        cur_wt = nxt_wt
                    if blk < NBLK - 1:
                        nxt_wt = wt_blk(blk + 1)
                if j % 4 == 0:
                    g = j // 4 + 2
                    if g * 4 < NJ:
                        wn_g(g)
                while wdone < min(NJ, j + 9):
                    wnorm_chunk(wdone)
                    wdone += 1
                    if wdone % 4 == 0 or wdone == NJ:
                        rn_fin((wdone - 1) // 4)

                if blk == 0:
                    def lhs(k, jj=jj):
                        if jj < 2:
                            return wt0a[:, k, jj * 128:(jj + 1) * 128]
                        return wt0b[:, k, (jj - 2) * 128:(jj - 1) * 128]
                else:
                    def lhs(k, jj=jj, cw=cur_wt):
                        return cw[:, k, jj * 128:(jj + 1) * 128]

                ot = outp.tile([128, b], BF16, tag="ot")
                for h in range(2):
                    ps = cpsum.tile([128, 1024], F32, tag="mmps")
                    for t in (2 * h, 2 * h + 1):
                        for k in range(kc):
                            nc.tensor.matmul(
                                out=ps[:, (t % 2) * 512:(t % 2) * 512 + 512],
                                lhsT=lhs(k),
                                rhs=eT[:, k, t * 512:(t + 1) * 512],
                                start=(k == 0),
                                stop=(k == kc - 1),
                            )
                    if h == 0:
                        nc.vector.tensor_scalar_mul(
                            out=ot[:, :1024], in0=ps[:, :], scalar1=rn[:, j:j + 1]
                        )
                    else:
                        nc.scalar.mul(
                            out=ot[:, 1024:], in_=ps[:, :], mul=rn[:, j:j + 1]
                        )
                nc.sync.dma_start(
                    out=outv[j * 128:(j + 1) * 128, :], in_=ot[:]
                )

                if j % 4 == 3 and wsel_done < mb:
                    phase_wsel(wsel_done)
                    wsel_done += 1
                if j == 66:
                    phi_block()

    nc.compile()
    return nc


_CACHE = {}


def _get_program():
    if "nc" not in _CACHE:
        _CACHE["nc"] = build_program()
    return _CACHE["nc"]


def make_in_maps(embeddings, labels, weight):
    emb = np.ascontiguousarray(np.asarray(embeddings, dtype=np.float32))
    w = np.asarray(weight, dtype=np.float32)
    labels_np = np.asarray(labels).astype(np.int64)
    ident = np.eye(128, dtype=np.float32)
    w_bf = w.astype(NPBF)
    in_maps = []
    for k in range(N_CORES):
        wn = np.zeros((CSP, D), NPBF)
        wn[:CS] = w_bf[k * CS:(k + 1) * CS]
        wT = np.ascontiguousarray(wn.T)
        own = (labels_np // CS) == k
        col = labels_np - k * CS
        goff = np.where(own, col, OOB).astype(np.int64)
        goff_arr = np.ascontiguousarray(
            goff.reshape(B // 128, 128).T.astype(np.int32)
        )
        in_maps.append(
            {"emb": emb, "wt": wT, "wn": wn, "goff": goff_arr, "ident": ident}
        )
    return in_maps


def _gather(results, labels):
    labels_np = np.asarray(labels).astype(np.int64)
    bidx = np.arange(B)
    fullT = np.empty((C, B), np.float32)
    for k in range(N_CORES):
        shard = np.asarray(results[k]["out"]).reshape(CSP, B)
        fullT[k * CS:(k + 1) * CS] = shard[:CS]
        # place the device-computed 64*phi values at the target positions
        tv = np.asarray(results[k]["tv"])  # [128, mb]
        own = (labels_np // CS) == k
        ob = bidx[own]
        fullT[labels_np[ob], ob] = tv[ob % 128, ob // 128]
    return fullT.T


def kernel(embeddings, labels, weight):
    nc = _get_program()
    in_maps = make_in_maps(embeddings, labels, weight)
    res = run_bass_kernel_spmd(nc, in_maps, core_ids=list(range(N_CORES)))
    return _gather(res.results, labels)


def kernel_profiled(embeddings, labels, weight, **kw):
    """Like kernel() but also returns the BassKernelResults (exec_time_ns)."""
    nc = _get_program()
    in_maps = make_in_maps(embeddings, labels, weight)
    res = run_bass_kernel_spmd(
        nc, in_maps, core_ids=list(range(N_CORES)), trace=True, **kw
    )
    return _gather(res.results, labels), res


# revision 16
# speedup vs baseline: 1.2050x; 1.2050x over previous
"""ArcMargin head (ArcFace) distributed over 8 TRN2 NeuronCores.

Strategy (classification / tensor parallel), v4 — transposed-output design:
  - weight [C, D] sharded along C (12500 classes/core, padded to 12544);
    embeddings + labels replicated.
  - The host uploads the weight shard twice in bf16 (same total bytes as one
    f32 copy): once pre-TRANSPOSED [D, CSP] so the TensorEngine needs no
    on-chip weight transposes, once natural [CSP, D] for the row-norm
    computation (free-axis Square+accum) and for the per-label row gather.
  - The device computes the TRANSPOSED logits block out[c, b] = 64 *
    (w_c . e_hat_b):  lhsT = wT tile (classes stationary), rhs = eT.  With
    classes on PSUM partitions, the weight normalization folds into PSUM
    evacuation as a free per-partition scalar multiply (alternating
    Vector/Scalar engines), and the output is written as bf16 (halves the
    dominant HBM traffic; rel-err budget 2e-2 >> bf16 noise).
  - ArcFace margin: only the single target element per row changes.  The
    2048 target cosines come from an indirect row gather of weight[labels]
    plus a fused normalize-dot on GpSimd; the phi values are DMA'd out as a
    tiny [128, 16] tensor and placed into the full output during the host
    unshard step (all math stays on device; the host only does indexing).
  - Ramp: embeddings load as 16 small DMAs ahead of everything, the first
    weight block is split so the first matmuls can start at ~7 us, and the
    embedding-norm pipeline is spread across Scalar/Vector/GpSimd.
"""

import math
import sys

import numpy as np
import ml_dtypes

for _p in ("/opt/trn_rl_repo",):
    if _p not in sys.path:
        sys.path.append(_p)

import concourse.bass as bass
import concourse.tile as tile
from concourse import bacc
from concourse import mybir
from concourse.bass_utils import run_bass_kernel_spmd

SCALE = 64.0
MARGIN = 0.5
COS_M = math.cos(MARGIN)
SIN_M = math.sin(MARGIN)
TH = math.cos(math.pi - MARGIN)
MM = math.sin(math.pi - MARGIN) * MARGIN

B, D, C = 2048, 512, 100000
N_CORES = 8
CS = C // N_CORES          # 12500 real classes per core
CSP = 12544                # padded classes per core (98 * 128)
NJ = CSP // 128            # 98 class chunks
CB = 1792                  # weight-block width (7 blocks x 14 chunks)
NBLK = CSP // CB           # 7
JPB = CB // 128            # 14 chunks per block
OOB = 1 << 30              # gather offset sentinel for "not my row"

NPBF = ml_dtypes.bfloat16

F32 = mybir.dt.float32
BF16 = mybir.dt.bfloat16
I32 = mybir.dt.int32
AF = mybir.ActivationFunctionType
ALU = mybir.AluOpType


def build_program(b=B, d=D, csp=CSP):
    """Build the (SPMD-uniform) single-core Bass program."""
    mb = b // 128          # 16 batch row-chunks
    kc = d // 128          # 4 contraction chunks
    nc = bacc.Bacc()

    emb_d = nc.declare_dram_parameter("emb", [b, d], F32, isOutput=False)
    wt_d = nc.declare_dram_parameter("wt", [d, csp], BF16, isOutput=False)
    wn_d = nc.declare_dram_parameter("wn", [csp, d], BF16, isOutput=False)
    goff_d = nc.declare_dram_parameter("goff", [128, mb], I32, isOutput=False)
    ident_d = nc.declare_dram_parameter("ident", [128, 128], F32, isOutput=False)
    # flat transposed output [c * B + b]
    out_d = nc.declare_dram_parameter("out", [csp * b, 1], BF16, isOutput=True)
    tv_d = nc.declare_dram_parameter("tv", [128, mb], F32, isOutput=True)

    with tile.TileContext(nc) as tc:
        with (
            tc.tile_pool(name="const", bufs=1) as constp,
            tc.tile_pool(name="persist", bufs=1) as persist,
            tc.tile_pool(name="eld", bufs=16) as eldp,
            tc.tile_pool(name="wtp", bufs=3) as wtp,
            tc.tile_pool(name="wnp", bufs=3) as wnp,
            tc.tile_pool(name="scr", bufs=2) as scrp,
            tc.tile_pool(name="smp", bufs=4) as smp,
            tc.tile_pool(name="outp", bufs=3) as outp,
            tc.tile_pool(name="tpsum", bufs=2, space="PSUM") as tpsum,
            tc.tile_pool(name="cpsum", bufs=3, space="PSUM") as cpsum,
        ):
            ident = constp.tile([128, 128], BF16)
            nc.gpsimd.dma_start(out=ident[:], in_=ident_d[:])  # SWDGE casts f32->bf16
            zb = constp.tile([128, 1], F32, tag="zb")
            nc.vector.memset(zb[:], 0.0)
            epsb = constp.tile([128, 1], F32, tag="epsb")
            nc.vector.memset(epsb[:], 1e-24)
            s2b = constp.tile([128, 1], F32, tag="s2b")
            nc.vector.memset(s2b[:], SCALE * SCALE)

            eT = persist.tile([128, kc, b], BF16)       # (64*e_hat)^T
            ebf_all = persist.tile([128, mb, d], BF16)  # 64*e_hat, natural layout
            nsq = persist.tile([128, NJ], F32)          # per-class sum(w^2)
            nrm = persist.tile([128, NJ], F32)
            rn = persist.tile([128, NJ], F32)           # 1/||w_c||
            svec = persist.tile([128, mb], F32)         # 64*cos(target)
            tval = persist.tile([128, mb], F32)         # 64*phi / else-branch
            gofft = persist.tile([128, mb], I32)

            outv = out_d[:].rearrange("(c b) o -> c (b o)", b=b)  # [csp, b]

            # ---------------- DMA helpers ----------------
            def eg_m(m):
                t = eldp.tile([128, 1, d], F32, tag="eg", name=f"eg_{m}")
                nc.sync.dma_start(
                    out=t[:, 0, :],
                    in_=emb_d[m * 128:(m + 1) * 128, :],
                )
                return t

            def wt_blk(blk):
                t = wtp.tile([128, kc, CB], BF16, tag="wt", name=f"wt_{blk}")
                nc.sync.dma_start(
                    out=t[:],
                    in_=wt_d[:, blk * CB:(blk + 1) * CB].rearrange(
                        "(k p) c -> p k c", p=128
                    ),
                )
                return t

            wn_tiles = {}

            def wn_g(g):
                r0 = g * 512
                ng = min(4, NJ - g * 4)
                t = wnp.tile([128, 4, d], BF16, tag="wn", name=f"wn_{g}")
                nc.sync.dma_start(
                    out=t[:, :ng, :],
                    in_=wn_d[r0:r0 + ng * 128, :].rearrange(
                        "(g2 p) dd -> p g2 dd", p=128
                    ),
                )
                wn_tiles[g] = t

            # ---------------- compute helpers ----------------
            def e_norm(m, eg):
                et = eg[:, 0, :]
                ssq = smp.tile([128, 1], F32, tag="ssqe")
                if m % 2 == 0:
                    sq = scrp.tile([128, d], BF16, tag="sqe")
                    nc.scalar.activation(
                        out=sq[:], in_=et, func=AF.Square, bias=zb[:],
                        accum_out=ssq[:],
                    )
                else:
                    sq = scrp.tile([128, d], BF16, tag="sqe1")
                    nc.vector.scalar_tensor_tensor(
                        out=sq[:], in0=et, scalar=1.0, in1=et,
                        op0=ALU.mult, op1=ALU.mult, accum_out=ssq[:],
                    )
                nrm_e = smp.tile([128, 1], F32, tag="nrme")
                nc.scalar.activation(out=nrm_e[:], in_=ssq[:], func=AF.Sqrt, bias=epsb[:])
                rec = smp.tile([128, 1], F32, tag="rece")
                nc.vector.reciprocal(out=rec[:], in_=nrm_e[:])
                # ebf = (e * (1/||e||)) * 64, fused two-scalar op (DVE only:
                # TensorScalarPtr is not available on the Pool engine)
                nc.vector.tensor_scalar(
                    out=ebf_all[:, m, :], in0=et, scalar1=rec[:], scalar2=SCALE,
                    op0=ALU.mult, op1=ALU.mult,
                )

            def e_transpose(m):
                for k in range(kc):
                    pt = tpsum.tile([128, 128], BF16)
                    nc.tensor.transpose(
                        out=pt[:], in_=ebf_all[:, m, k * 128:(k + 1) * 128],
                        identity=ident[:],
                    )
                    if (m * kc + k) % 2 == 0:
                        nc.scalar.copy(
                            out=eT[:, k, m * 128:(m + 1) * 128], in_=pt[:]
                        )
                    else:
                        nc.vector.tensor_copy(
                            out=eT[:, k, m * 128:(m + 1) * 128], in_=pt[:]
                        )

            def wnorm_chunk(c):
                sq = scrp.tile([128, d], BF16, tag="sqw")
                nc.scalar.activation(
                    out=sq[:], in_=wn_tiles[c // 4][:, c % 4, :], func=AF.Square,
                    bias=zb[:], accum_out=nsq[:, c:c + 1],
                )

            def rn_fin(g):
                s0 = g * 4
                s1 = min(s0 + 4, NJ)
                nc.scalar.activation(
                    out=nrm[:, s0:s1], in_=nsq[:, s0:s1], func=AF.Sqrt, bias=epsb[:]
                )
                nc.vector.reciprocal(out=rn[:, s0:s1], in_=nrm[:, s0:s1])

            def phase_wsel(m):
                # gather this chunk's owned target weight rows from the shard
                wsld = scrp.tile([128, d], BF16, tag="wsld", name=f"ws_{m}")
                nc.gpsimd.indirect_dma_start(
                    out=wsld[:],
                    out_offset=None,
                    in_=wn_d[:],
                    in_offset=bass.IndirectOffsetOnAxis(
                        ap=gofft[:, m:m + 1], axis=0
                    ),
                    bounds_check=csp - 1,
                    oob_is_err=False,
                )
                sq = scrp.tile([128, d], BF16, tag="sqs")
                ssq = smp.tile([128, 1], F32, tag="ssqs")
                nc.scalar.activation(
                    out=sq[:], in_=wsld[:], func=AF.Square, bias=zb[:],
                    accum_out=ssq[:],
                )
                nrm_s = smp.tile([128, 1], F32, tag="nrms")
                nc.scalar.activation(out=nrm_s[:], in_=ssq[:], func=AF.Sqrt, bias=epsb[:])
                rec = smp.tile([128, 1], F32, tag="recs")
                nc.vector.reciprocal(out=rec[:], in_=nrm_s[:])
                # dot = sum(wsld * ebf), then svec = dot * (1/||w||)
                ttr = scrp.tile([128, d], BF16, tag="ttr")
                dotm = smp.tile([128, 1], F32, tag="dotm")
                nc.vector.scalar_tensor_tensor(
                    out=ttr[:], in0=wsld[:], scalar=1.0, in1=ebf_all[:, m, :],
                    op0=ALU.mult, op1=ALU.mult, accum_out=dotm[:],
                )
                nc.vector.tensor_tensor(
                    out=svec[:, m:m + 1], in0=dotm[:], in1=rec[:], op=ALU.mult
                )

            def phi_block():
                s2 = smp.tile([128, mb], F32, tag="s2")
                nc.scalar.activation(
                    out=s2[:], in_=svec[:], func=AF.Square, bias=zb[:]
                )
                rl = smp.tile([128, mb], F32, tag="rl")
                nc.scalar.activation(
                    out=rl[:], in_=s2[:], func=AF.Relu, bias=s2b[:], scale=-1.0
                )
                sn = smp.tile([128, mb], F32, tag="sn")
                nc.scalar.activation(out=sn[:], in_=rl[:], func=AF.Sqrt, bias=zb[:])
                pc = smp.tile([128, mb], F32, tag="pc")
                nc.vector.tensor_scalar_mul(out=pc[:], in0=svec[:], scalar1=COS_M)
                smt = smp.tile([128, mb], F32, tag="smt")
                nc.vector.tensor_scalar_mul(out=smt[:], in0=sn[:], scalar1=SIN_M)
                ph = smp.tile([128, mb], F32, tag="ph")
                nc.vector.tensor_tensor(
                    out=ph[:], in0=pc[:], in1=smt[:], op=ALU.subtract
                )
                eb = smp.tile([128, mb], F32, tag="eb")
                nc.vector.tensor_scalar_add(
                    out=eb[:], in0=svec[:], scalar1=-SCALE * MM
                )
                mk = smp.tile([128, mb], mybir.dt.uint8, tag="mk")
                nc.vector.tensor_scalar(
                    out=mk[:], in0=svec[:], scalar1=SCALE * TH, scalar2=None,
                    op0=ALU.is_gt,
                )
                nc.vector.select(out=tval[:], mask=mk[:], on_true=ph[:], on_false=eb[:])
                nc.sync.dma_start(out=tv_d[:], in_=tval[:])

            # ---------------- prologue (DMA order = ring order) ----------------
            eg_tiles = [None] * mb
            for m in range(8):
                eg_tiles[m] = eg_m(m)
            wn_g(0)
            # split first weight block: j0/j1 lhsT available early
            wt0a = wtp.tile([128, kc, 256], BF16, tag="wt0a")
            nc.sync.dma_start(
                out=wt0a[:],
                in_=wt_d[:, 0:256].rearrange("(k p) c -> p k c", p=128),
            )
            for m in range(8, mb):
                eg_tiles[m] = eg_m(m)
            wt0b = wtp.tile([128, kc, CB - 256], BF16, tag="wt0b")
            nc.sync.dma_start(
                out=wt0b[:],
                in_=wt_d[:, 256:CB].rearrange("(k p) c -> p k c", p=128),
            )
            wn_g(1)
            nc.sync.dma_start(out=gofft[:], in_=goff_d[:])

            wdone = 0
            for m in range(mb):
                e_norm(m, eg_tiles[m])
                e_transpose(m)
                if m % 4 == 3 and wdone < 8:
                    wnorm_chunk(wdone)
                    wnorm_chunk(wdone + 1)
                    wdone += 2
                    if wdone % 4 == 0:
                        rn_fin(wdone // 4 - 1)

            # ---------------- main loop over class chunks ----------------
            cur_wt = None
            nxt_wt = None
            wsel_done = 0
            for j in range(NJ):
                blk, jj = divmod(j, JPB)
                if jj == 0:
                    if blk > 0:
                        cur_wt = nxt_wt
                    if blk < NBLK - 1:
                        nxt_wt = wt_blk(blk + 1)
                if j % 4 == 0:
                    g = j // 4 + 2
                    if g * 4 < NJ:
                        wn_g(g)
                while wdone < min(NJ, j + 9):
                    wnorm_chunk(wdone)
                    wdone += 1
                    if wdone % 4 == 0 or wdone == NJ:
                        rn_fin((wdone - 1) // 4)

                if blk == 0:
                    def lhs(k, jj=jj):
                        if jj < 2:
                            return wt0a[:, k, jj * 128:(jj + 1) * 128]
                        return wt0b[:, k, (jj - 2) * 128:(jj - 1) * 128]
                else:
                    def lhs(k, jj=jj, cw=cur_wt):
                        return cw[:, k, jj * 128:(jj + 1) * 128]

                ot = outp.tile([128, b], BF16, tag="ot")
                for h in range(2):
                    ps = cpsum.tile([128, 1024], F32, tag="mmps")
                    for t in (2 * h, 2 * h + 1):
                        for k in range(kc):
                            nc.tensor.matmul(
                                out=ps[:, (t % 2) * 512:(t % 2) * 512 + 512],
                                lhsT=lhs(k),
                                rhs=eT[:, k, t * 512:(t + 1) * 512],
                                start=(k == 0),
                                stop=(k == kc - 1),
                            )
                    if h == 0:
                        nc.vector.tensor_scalar_mul(
                            out=ot[:, :1024], in0=ps[:, :], scalar1=rn[:, j:j + 1]
                        )
                    else:
                        nc.scalar.mul(
                            out=ot[:, 1024:], in_=ps[:, :], mul=rn[:, j:j + 1]
                        )
                nc.sync.dma_start(
                    out=outv[j * 128:(j + 1) * 128, :], in_=ot[:]
                )

                if j % 6 == 3 and wsel_done < mb:
                    phase_wsel(wsel_done)
                    wsel_done += 1
                if j == 94:
                    phi_block()

    nc.compile()
    return nc


_CACHE = {}


def _get_program():
    if "nc" not in _CACHE:
        _CACHE["nc"] = build_program()
    return _CACHE["nc"]


def make_in_maps(embeddings, labels, weight):
    emb = np.ascontiguousarray(np.asarray(embeddings, dtype=np.float32))
    w = np.asarray(weight, dtype=np.float32)
    labels_np = np.asarray(labels).astype(np.int64)
    ident = np.eye(128, dtype=np.float32)
    w_bf = w.astype(NPBF)
    in_maps = []
    for k in range(N_CORES):
        wn = np.zeros((CSP, D), NPBF)
        wn[:CS] = w_bf[k * CS:(k + 1) * CS]
        wT = np.ascontiguousarray(wn.T)
        own = (labels_np // CS) == k
        col = labels_np - k * CS
        goff = np.where(own, col, OOB).astype(np.int64)
        goff_arr = np.ascontiguousarray(
            goff.reshape(B // 128, 128).T.astype(np.int32)
        )
        in_maps.append(
            {"emb": emb, "wt": wT, "wn": wn, "goff": goff_arr, "ident": ident}
        )
    return in_maps


def _gather(results, labels):
    labels_np = np.asarray(labels).astype(np.int64)
    bidx = np.arange(B)
    fullT = np.empty((C, B), np.float32)
    for k in range(N_CORES):
        shard = np.asarray(results[k]["out"]).reshape(CSP, B)
        fullT[k * CS:(k + 1) * CS] = shard[:CS]
        # place the device-computed 64*phi values at the target positions
        tv = np.asarray(results[k]["tv"])  # [128, mb]
        own = (labels_np // CS) == k
        ob = bidx[own]
        fullT[labels_np[ob], ob] = tv[ob % 128, ob // 128]
    return fullT.T


def kernel(embeddings, labels, weight):
    nc = _get_program()
    in_maps = make_in_maps(embeddings, labels, weight)
    res = run_bass_kernel_spmd(nc, in_maps, core_ids=list(range(N_CORES)))
    return _gather(res.results, labels)


def kernel_profiled(embeddings, labels, weight, **kw):
    """Like kernel() but also returns the BassKernelResults (exec_time_ns)."""
    nc = _get_program()
    in_maps = make_in_maps(embeddings, labels, weight)
    res = run_bass_kernel_spmd(
        nc, in_maps, core_ids=list(range(N_CORES)), trace=True, **kw
    )
    return _gather(res.results, labels), res
